# revision 15
# baseline (speedup 1.0000x reference)
"""DecodeDetections kernel for Trainium2 (Bass/Tile), 8-core data-parallel.

Full input y_pred [64, 8732, 33] f32 -> output [64, 200, 6] f32.
Each of the 8 NeuronCores handles 8 batch items ("tokens").

Per-core pipeline (partition p = 16t + i holds boxes [546i, 546(i+1)) of
token t):
  1. Strided DMA extracts score channels 1..21 per box directly from DRAM
     into S21 [128, 546*21] (ch 21 is a fake "score" fixed up later).
  2. DVE tensor_reduce(max) over in-box triples -> block maxes BM
     [128, 3824] (3822 real blocks of 3 = 7 per box; pair-block j=6 fixed
     with a 2-wide reduce that excludes the fake channel).
  3. gpsimd topk (vocab 61184) -> exact top-256 blocks per token.
  4. Indirect-gather each winning block's 3 contiguous y elements into
     C [128, 3640]; mask the fake lane of j=6 blocks.
  5. gpsimd topk (vocab 58240) over C -> exact top-256 score values.
  6. Indirect-gather winner block ids, derive (cls, box); gather winner
     rows; decode SSD boxes (Taylor exp).
  7. Exact rank via +-2 tie window on [8, 260] (value desc, m asc);
     blend rows by rank delta and write out[t*200+j] with a reversed-
     stride DMA (no indirect scatter).
"""

import os
import sys

for _p in ("/opt/trn_rl_repo", "/root/.axon_site/_ro/trn_rl_repo"):
    if os.path.isdir(_p) and _p not in sys.path:
        sys.path.insert(0, _p)

import numpy as np

import concourse.bass as bass
import concourse.bacc as bacc
import concourse.bass_isa as bass_isa
import concourse.mybir as mybir
import concourse.tile as tile
from concourse.bass_types import AP
from concourse.bass_utils import run_bass_kernel_spmd

# problem constants
B = 64
NBOX = 8732
NCH = 33
TOPK = 200
NCORES = 8
TPC = 8            # tokens (batch items) per core
NBB = 546          # boxes per partition
NBP = 8736         # padded boxes per token in DRAM
NELEM = TPC * NBP * NCH
SW = NBB * 21      # 11466 score+fake cols per partition
NBLK = NBB * 7     # 3822 blocks per partition
BMW = 3824         # padded block cols (vocab 61184)
CW = 3640          # candidate tile cols (vocab 58240)
IMG = 512.0

f32 = mybir.dt.float32
u32 = mybir.dt.uint32
i32 = mybir.dt.int32


def _topk(nc, out_ap, in_ap, tokens, vocab, k=256):
    _in = nc.gpsimd.lower_ap(in_ap, for_isa=True)
    _out = nc.gpsimd.lower_ap(out_ap, for_isa=True)
    return nc.gpsimd.add_instruction(
        bass_isa.InstTopk(name=f"I-{nc.next_id()}", ins=[_in], outs=[_out],
                          _tokens=tokens, _n=vocab, _k=k))


class _Helper:
    """Float-exact integer div/mod on [128, W] f32 tiles."""

    def __init__(self, nc, pool, w):
        self.nc, self.pool, self.w = nc, pool, w
        self.t1 = pool.tile([128, w], f32, name="hlp_t1")
        self.ti = pool.tile([128, w], i32, name="hlp_ti")
        self.t2 = pool.tile([128, w], f32, name="hlp_t2")

    def fdiv(self, out, in_, d):
        nc = self.nc
        nc.vector.tensor_scalar(self.t1[:], in_, float((1 + 2.0 ** -20) / d),
                                scalar2=None, op0=mybir.AluOpType.mult)
        nc.vector.tensor_copy(self.ti[:], self.t1[:])
        nc.vector.tensor_copy(out, self.ti[:])
        nc.vector.tensor_scalar(self.t1[:], out, float(d),
                                scalar2=None, op0=mybir.AluOpType.mult)
        nc.vector.tensor_tensor(self.t2[:], self.t1[:], in_,
                                op=mybir.AluOpType.is_gt)
        nc.vector.tensor_tensor(out, out, self.t2[:],
                                op=mybir.AluOpType.subtract)

    def fmod(self, out, in_, quot, d):
        nc = self.nc
        nc.vector.tensor_scalar(self.t1[:], quot, float(d),
                                scalar2=None, op0=mybir.AluOpType.mult)
        nc.vector.tensor_tensor(out, in_, self.t1[:],
                                op=mybir.AluOpType.subtract)


def build_kernel():
    nc = bacc.Bacc("TRN2", target_bir_lowering=False, debug=False)
    y = nc.dram_tensor("y", [TPC * NBP, NCH], f32, kind="ExternalInput")
    out = nc.dram_tensor("out", [TPC * TOPK, 6], f32, kind="ExternalOutput")

    TS = mybir.AluOpType
    with tile.TileContext(nc) as tc:
        with tc.tile_pool(name="sbuf", bufs=1) as pool, \
             tc.tile_pool(name="dram", bufs=1, space="DRAM") as dpool:

            # ---- per-partition constants (run while DMAs stream in) ----
            pidx = pool.tile([128, 1], i32)
            nc.gpsimd.iota(pidx[:], pattern=[[0, 1]], base=0,
                           channel_multiplier=1)
            pf = pool.tile([128, 1], f32)
            nc.vector.tensor_copy(pf[:], pidx[:])
            h1 = _Helper(nc, pool, 1)
            tf = pool.tile([128, 1], f32)
            h1.fdiv(tf[:], pf[:], 16)
            t16 = pool.tile([128, 1], f32)
            nc.vector.tensor_scalar(t16[:], tf[:], 16.0, scalar2=None,
                                    op0=TS.mult)
            t256 = pool.tile([128, 1], f32)
            nc.vector.tensor_scalar(t256[:], tf[:], 256.0, scalar2=None,
                                    op0=TS.mult)
            t8736 = pool.tile([128, 1], f32)
            nc.vector.tensor_scalar(t8736[:], tf[:], float(NBP), scalar2=None,
                                    op0=TS.mult)

            BMt = pool.tile([128, BMW], f32)
            nc.vector.memset(BMt[:, NBLK:BMW], 0.0)
            C = pool.tile([128, CW], f32)
            nc.vector.memset(C[:], 0.0)

            # ---- extraction + per-chunk block max ----
            yv = y[:].rearrange("(p b) c -> p b c", p=128)
            bchunks = (0, 91, 182, 273, 364, 455, 546)
            engs = (nc.sync, nc.scalar, nc.gpsimd,
                    nc.sync, nc.scalar, nc.gpsimd)
            Sc = []
            for j in range(6):
                b0, b1 = bchunks[j], bchunks[j + 1]
                St = pool.tile([128, (b1 - b0) * 21], f32, name=f"S{j}")
                Sc.append((St, b0, b1))
                with nc.named_scope(f"sload{j}"):
                    engs[j].dma_start(
                        St[:].rearrange("p (b c) -> p b c", c=21),
                        yv[:, b0:b1, 1:22])
            for j, (St, b0, b1) in enumerate(Sc):
                nb = b1 - b0
                with nc.named_scope(f"bmax{j}"):
                    nc.vector.tensor_reduce(
                        BMt[:, b0 * 7:b1 * 7],
                        St[:].rearrange("p (x three) -> p x three", three=3),
                        axis=mybir.AxisListType.X, op=TS.max)
                    nc.vector.tensor_reduce(
                        BMt[:, b0 * 7:b1 * 7].rearrange(
                            "p (b seven) -> p b seven", seven=7)[:, :, 6],
                        St[:].rearrange("p (b c) -> p b c", c=21)[:, :, 18:20],
                        axis=mybir.AxisListType.X, op=TS.max)

            tk1 = pool.tile([128, 32], u32)
            with nc.named_scope("tk1"):
                _topk(nc, tk1[:], BMt[:], tokens=TPC, vocab=BMW * 16)

            # ---- winner-block math -> y offsets of the 3 elements ----
            h16 = _Helper(nc, pool, 16)
            r1f = pool.tile([128, 16], f32)
            nc.vector.tensor_copy(r1f[:], tk1[:, 16:32])
            i1 = pool.tile([128, 16], f32)
            x1 = pool.tile([128, 16], f32)
            b1t = pool.tile([128, 16], f32)
            j1 = pool.tile([128, 16], f32)
            h16.fdiv(i1[:], r1f[:], BMW)
            h16.fmod(x1[:], r1f[:], i1[:], BMW)
            h16.fdiv(b1t[:], x1[:], 7)
            h16.fmod(j1[:], x1[:], b1t[:], 7)
            u = pool.tile([128, 16], f32)
            o1 = pool.tile([128, 16], f32)
            nc.vector.tensor_scalar(u[:], i1[:], t16[:, 0:1], scalar2=None,
                                    op0=TS.add)
            nc.vector.tensor_scalar(u[:], u[:], float(NBB), scalar2=None,
                                    op0=TS.mult)
            nc.vector.tensor_tensor(u[:], u[:], b1t[:], op=TS.add)
            nc.vector.tensor_scalar(u[:], u[:], 33.0, scalar2=None,
                                    op0=TS.mult)
            nc.vector.tensor_scalar(o1[:], j1[:], 3.0, scalar2=1.0,
                                    op0=TS.mult, op1=TS.add)
            nc.vector.tensor_tensor(o1[:], o1[:], u[:], op=TS.add)
            offsu = pool.tile([128, 16], u32)
            nc.vector.tensor_copy(offsu[:], o1[:])

            # fake-lane indicator (j == 6) for the pair blocks
            isj6 = pool.tile([128, 16], f32)
            nc.vector.tensor_scalar(isj6[:], j1[:], 5.5, scalar2=None,
                                    op0=TS.is_gt)

            # bounce (offset, isj6) through DRAM reversed within each token
            # group so tk2 scans candidates in descending-value order
            # (ascending order forces a heap insert per element).
            od = dpool.tile([128 * 16, 2], f32)
            nc.sync.dma_start(
                od[:, 0:1].rearrange("(p c) o -> p (c o)", p=128), o1[:])
            nc.sync.dma_start(
                od[:, 1:2].rearrange("(p c) o -> p (c o)", p=128), isj6[:])
            odR = pool.tile([128, 16, 2], f32)
            odsrc = AP(tensor=od[:].tensor, offset=255 * 2,
                       ap=[[512, 8], [-32, 16], [-2, 16], [1, 2]])
            nc.sync.dma_start(odR[:], odsrc)
            offsRu = pool.tile([128, 16], u32)
            nc.vector.tensor_copy(offsRu[:], odR[:, :, 0])

            # ---- gather candidate triples (reversed slot order) ----
            ysrc = AP(tensor=y[:].tensor, offset=0,
                      ap=[[1, NELEM], [1, 3]])
            with nc.named_scope("candgather"):
                for k in range(16):
                    nc.gpsimd.indirect_dma_start(
                        out=C[:, 3 * k:3 * k + 3], out_offset=None, in_=ysrc,
                        in_offset=bass.IndirectOffsetOnAxis(
                            ap=offsRu[:, k:k + 1], axis=1),
                        bounds_check=NELEM - 3, oob_is_err=False)
            # zero the fake lane of pair blocks: mult by (1 - isj6R)
            negj6 = pool.tile([128, 16], f32)
            nc.vector.tensor_scalar(negj6[:], odR[:, :, 1], -1.0, scalar2=1.0,
                                    op0=TS.mult, op1=TS.add)
            cview = C[:, 0:48].rearrange("p (k three) -> p k three",
                                         three=3)[:, :, 2]
            nc.vector.tensor_tensor(cview, cview, negj6[:], op=TS.mult)

            # per-slot metadata table (original slot order):
            # n_tok = i1*546 + b, yrow = t*8736 + n_tok,
            # basecl = 3*j1 + 1, mm0 = basecl*8732 + n_tok
            ntok = pool.tile([128, 16], f32)
            nc.vector.tensor_scalar(ntok[:], i1[:], float(NBB), scalar2=None,
                                    op0=TS.mult)
            nc.vector.tensor_tensor(ntok[:], ntok[:], b1t[:], op=TS.add)
            yrowS = pool.tile([128, 16], f32)
            nc.vector.tensor_scalar(yrowS[:], ntok[:], t8736[:, 0:1],
                                    scalar2=None, op0=TS.add)
            yrowU = pool.tile([128, 16], u32)
            nc.vector.tensor_copy(yrowU[:], yrowS[:])
            basecl = pool.tile([128, 16], f32)
            nc.vector.tensor_scalar(basecl[:], j1[:], 3.0, scalar2=1.0,
                                    op0=TS.mult, op1=TS.add)
            mm0 = pool.tile([128, 16], f32)
            nc.vector.tensor_scalar(mm0[:], basecl[:], float(NBOX),
                                    scalar2=None, op0=TS.mult)
            nc.vector.tensor_tensor(mm0[:], mm0[:], ntok[:], op=TS.add)
            Rd2 = dpool.tile([128 * 16, 4], f32)
            nc.sync.dma_start(
                Rd2[:, 0:1].rearrange("(p c) o -> p (c o)", p=128), yrowS[:])
            nc.sync.dma_start(
                Rd2[:, 1:2].rearrange("(p c) o -> p (c o)", p=128), mm0[:])
            nc.sync.dma_start(
                Rd2[:, 2:3].rearrange("(p c) o -> p (c o)", p=128), basecl[:])
            nc.sync.dma_start(
                Rd2[:, 3:4].bitcast(u32).rearrange("(p c) o -> p (c o)",
                                                   p=128), yrowU[:])

            tk2 = pool.tile([128, 32], u32)
            with nc.named_scope("tk2"):
                _topk(nc, tk2[:], C[:], tokens=TPC, vocab=CW * 16)

            # ---- winner math: position in C -> block slot -> block id ----
            v2f = pool.tile([128, 16], f32)
            nc.vector.tensor_copy(v2f[:], tk2[:, 16:32])
            i2 = pool.tile([128, 16], f32)
            c2 = pool.tile([128, 16], f32)
            k2 = pool.tile([128, 16], f32)
            w2 = pool.tile([128, 16], f32)
            h16.fdiv(i2[:], v2f[:], CW)
            h16.fmod(c2[:], v2f[:], i2[:], CW)
            h16.fdiv(k2[:], c2[:], 3)
            h16.fmod(w2[:], c2[:], k2[:], 3)
            # original slot = (15-i2, 15-k2): F = t256 + 255 - (16*i2 + k2)
            Ff = pool.tile([128, 16], f32)
            nc.vector.tensor_scalar(Ff[:], i2[:], 16.0, scalar2=None,
                                    op0=TS.mult)
            nc.vector.tensor_tensor(Ff[:], Ff[:], k2[:], op=TS.add)
            nc.vector.tensor_scalar(Ff[:], Ff[:], -1.0, scalar2=255.0,
                                    op0=TS.mult, op1=TS.add)
            nc.vector.tensor_scalar(Ff[:], Ff[:], t256[:, 0:1], scalar2=None,
                                    op0=TS.add)
            Fu = pool.tile([128, 16], u32)
            nc.vector.tensor_copy(Fu[:], Ff[:])

            mt = pool.tile([128, 16, 4], f32)
            with nc.named_scope("vgather"):
                for k in range(16):
                    nc.gpsimd.indirect_dma_start(
                        out=mt[:, k, :], out_offset=None, in_=Rd2[:],
                        in_offset=bass.IndirectOffsetOnAxis(
                            ap=Fu[:, k:k + 1], axis=0),
                        bounds_check=128 * 16 - 1, oob_is_err=False)

            # ---- winner fields from metadata + w ----
            cl = pool.tile([128, 16], f32)
            nc.vector.tensor_tensor(cl[:], mt[:, :, 2], w2[:], op=TS.add)
            m_ = pool.tile([128, 16], f32)
            nc.vector.tensor_scalar(m_[:], w2[:], float(NBOX), scalar2=None,
                                    op0=TS.mult)
            nc.vector.tensor_tensor(m_[:], m_[:], mt[:, :, 1], op=TS.add)
            conf = pool.tile([128, 16], f32)
            nc.vector.tensor_copy(conf[:], tk2[:, 0:16].bitcast(f32))

            # ---- rank path staging (sync queue; overlaps row gather) ----
            vmd = dpool.tile([2048, 2], f32)
            nc.sync.dma_start(
                vmd[:, 0:1].rearrange("(p c) o -> p (c o)", p=128), conf[:])
            nc.sync.dma_start(
                vmd[:, 1:2].rearrange("(p c) o -> p (c o)", p=128), m_[:])
            W = 2
            Vs = pool.tile([8, 256 + 2 * W], f32)
            Ms = pool.tile([8, 256 + 2 * W], f32)
            nc.vector.memset(Vs[:], -1.0)
            nc.vector.memset(Ms[:], 0.0)
            nc.sync.dma_start(
                Vs[0:8, W:W + 256],
                vmd[:, 0:1].rearrange("(t q) o -> t (q o)", t=8))
            nc.sync.dma_start(
                Ms[0:8, W:W + 256],
                vmd[:, 1:2].rearrange("(t q) o -> t (q o)", t=8))

            # ---- row gather (offsets come pre-converted via metadata) ----
            enc = pool.tile([128, 16, NCH], f32)
            with nc.named_scope("rowgather"):
                for k in range(16):
                    nc.gpsimd.indirect_dma_start(
                        out=enc[:, k, :], out_offset=None, in_=y[:],
                        in_offset=bass.IndirectOffsetOnAxis(
                            ap=mt[:, k, 3:4].bitcast(u32), axis=0),
                        bounds_check=TPC * NBP - 1, oob_is_err=False)

            # ---- rank delta DL on [8, 256] (runs on DVE during gather) ----
            Vc = Vs[:, W:W + 256]
            Mc = Ms[:, W:W + 256]
            DL = pool.tile([8, 256], f32)
            nc.vector.memset(DL[:], 0.0)
            eq = pool.tile([8, 256], f32)
            lt = pool.tile([8, 256], f32)
            for d in (1, 2, -1, -2):
                Vd = Vs[:, W + d:W + d + 256]
                Md = Ms[:, W + d:W + d + 256]
                nc.vector.tensor_tensor(eq[:], Vc, Vd, op=TS.is_equal)
                if d > 0:
                    nc.vector.tensor_tensor(DL[:], DL[:], eq[:],
                                            op=TS.subtract)
                nc.vector.tensor_tensor(lt[:], Md, Mc, op=TS.is_lt)
                nc.vector.tensor_tensor(lt[:], lt[:], eq[:], op=TS.mult)
                nc.vector.tensor_tensor(DL[:], DL[:], lt[:], op=TS.add)
            DLp = pool.tile([8, 256 + 2 * W], f32)
            nc.vector.memset(DLp[:], 99.0)
            nc.vector.tensor_copy(DLp[:, W:W + 256], DL[:])

            # ---- box decode (Taylor exp, identical to reference math) ----
            import math as _math
            EXP_C = [1.0 / _math.factorial(kk) for kk in range(11)]

            NW = 16

            def ch(k):
                return enc[:, :, 21 + k]

            cx = pool.tile([128, NW], f32)
            cy = pool.tile([128, NW], f32)
            we = pool.tile([128, NW], f32)
            he = pool.tile([128, NW], f32)
            nc.vector.tensor_tensor(cx[:], ch(0), ch(8), op=TS.mult)
            nc.vector.tensor_tensor(cx[:], cx[:], ch(6), op=TS.mult)
            nc.vector.tensor_tensor(cx[:], cx[:], ch(4), op=TS.add)
            nc.vector.tensor_tensor(cy[:], ch(1), ch(9), op=TS.mult)
            nc.vector.tensor_tensor(cy[:], cy[:], ch(7), op=TS.mult)
            nc.vector.tensor_tensor(cy[:], cy[:], ch(5), op=TS.add)
            nc.vector.tensor_tensor(we[:], ch(2), ch(10), op=TS.mult)
            nc.vector.tensor_tensor(he[:], ch(3), ch(11), op=TS.mult)
            xe = pool.tile([128, 2 * NW], f32)
            nc.vector.tensor_copy(xe[:, 0:NW], we[:])
            nc.vector.tensor_copy(xe[:, NW:2 * NW], he[:])
            acc = pool.tile([128, 2 * NW], f32)
            nc.vector.memset(acc[:], EXP_C[10])
            for kk in range(9, -1, -1):
                nc.vector.tensor_tensor(acc[:], acc[:], xe[:], op=TS.mult)
                nc.vector.tensor_scalar(acc[:], acc[:], EXP_C[kk],
                                        scalar2=None, op0=TS.add)
            nc.vector.tensor_tensor(we[:], acc[:, 0:NW], ch(6), op=TS.mult)
            nc.vector.tensor_tensor(he[:], acc[:, NW:2 * NW], ch(7),
                                    op=TS.mult)

            R6 = pool.tile([128, 16, 6], f32)
            nc.vector.tensor_copy(R6[:, :, 0], cl[:])
            nc.vector.tensor_copy(R6[:, :, 1], conf[:])
            cxs = pool.tile([128, NW], f32)
            whs = pool.tile([128, NW], f32)
            nc.vector.tensor_scalar(cxs[:], cx[:], IMG, scalar2=None,
                                    op0=TS.mult)
            nc.vector.tensor_scalar(whs[:], we[:], IMG / 2, scalar2=None,
                                    op0=TS.mult)
            nc.vector.tensor_tensor(R6[:, :, 2], cxs[:], whs[:],
                                    op=TS.subtract)
            nc.vector.tensor_tensor(R6[:, :, 4], cxs[:], whs[:],
                                    op=TS.add)
            nc.vector.tensor_scalar(cxs[:], cy[:], IMG, scalar2=None,
                                    op0=TS.mult)
            nc.vector.tensor_scalar(whs[:], he[:], IMG / 2, scalar2=None,
                                    op0=TS.mult)
            nc.vector.tensor_tensor(R6[:, :, 3], cxs[:], whs[:],
                                    op=TS.subtract)
            nc.vector.tensor_tensor(R6[:, :, 5], cxs[:], whs[:],
                                    op=TS.add)

            # ---- blend rows by rank delta, write reversed ----
            rows6d = dpool.tile([2048, 6], f32)
            nc.sync.dma_start(
                rows6d[:].rearrange("(p k) c -> p (k c)", p=128),
                R6[:].rearrange("p k c -> p (k c)"))
            Rp = pool.tile([8, 260 * 6], f32)
            nc.vector.memset(Rp[:, 0:12], 0.0)
            nc.vector.memset(Rp[:, 1548:1560], 0.0)
            nc.sync.dma_start(
                Rp[:, 12:1548].rearrange("t (q c) -> t q c", c=6),
                rows6d[:].rearrange("(t q) c -> t q c", t=8))
            G = pool.tile([8, 256 * 6], f32)
            nc.vector.memset(G[:], 0.0)
            Q0 = 54  # ranks > 201 are never written to the output
            NQ = 256 - Q0
            cmp = pool.tile([8, NQ], f32)
            cmpi = pool.tile([8, NQ], i32)
            for e in (-2, -1, 0, 1, 2):
                nc.vector.tensor_scalar(cmp[:],
                                        DLp[:, W + Q0 + e:W + e + 256],
                                        float(e), scalar2=None,
                                        op0=TS.is_equal)
                nc.vector.tensor_copy(cmpi[:], cmp[:])
                nc.vector.copy_predicated(
                    G[:, Q0 * 6:].rearrange("t (q c) -> t q c", c=6),
                    cmpi[:].rearrange("t (q o) -> t q o", o=1).to_broadcast(
                        [8, NQ, 6]),
                    Rp[:, (W + Q0 + e) * 6:(W + e + 256) * 6].rearrange(
                        "t (q c) -> t q c", c=6))

            gsrc = AP(tensor=G.tensor, offset=255 * 6,
                      ap=[[256 * 6, 8], [-6, 200], [1, 6]])
            with nc.named_scope("outw"):
                nc.sync.dma_start(
                    out[:].rearrange("(t j) c -> t j c", t=8), gsrc)

    nc.finalize()
    return nc


_NC = None


def kernel(y_pred: np.ndarray, _trace: bool = False) -> np.ndarray:
    global _NC
    y_pred = np.asarray(y_pred, dtype=np.float32)
    assert y_pred.shape == (B, NBOX, NCH)
    if _NC is None:
        _NC = build_kernel()
    in_maps = []
    for c in range(NCORES):
        sl = y_pred[c * TPC:(c + 1) * TPC]          # [8, 8732, 33]
        ypad = np.zeros((TPC, NBP, NCH), np.float32)
        ypad[:, :NBOX] = sl
        in_maps.append({"y": ypad.reshape(TPC * NBP, NCH)})
    res = run_bass_kernel_spmd(_NC, in_maps, core_ids=list(range(NCORES)),
                               trace=_trace)
    kernel._last_results = res
    outs = [r["out"].reshape(TPC, TOPK, 6) for r in res.results]
    return np.concatenate(outs, axis=0)
